# revision 18
# baseline (speedup 1.0000x reference)
"""Trainium2 Bass kernel for nn_AttentionPooling (segment-softmax attention pooling).

Math restructuring (vs the reference):
  scores[n,h] = (x @ Wk.T + bk) . pool_query  * scale  ==  x @ As + c0
      with As[j,h] = scale * sum_d Wk[h*HD+d, j] * pq[h,d]   (tiny [256,8])
  e = exp(scores)            (no max-subtraction needed; |scores| < ~4)
  z[c,h]   = sum_{n in c} e[n,h]
  U[c,h,:] = sum_{n in c} e[n,h] * x[n,:]
  T = U / z;  ssum[c, h*HD:(h+1)*HD] = T[c,h,:] @ Wv_h.T + bv_h   (softmax weights
      sum to 1 per segment, so the bias term is exact)
  pooled = ssum / max(cnt,1);  table = pooled @ Wo.T + bo;  out = table[seg]

Device work (8 cores, SPMD over cluster-sharded, cluster-sorted nodes):
  pass A: scores/e from feature-major x  (PE matmul vs As, ACT exp, PE transpose)
  pass B: U and z via one fp32 matmul per 128-node tile: stationary operand is a
      "weighted indicator" wind[n, slot*8+h] = (segw[n]==slot) * e[n,h] built on
      DVE; moving operand is x (augmented with a ones column so z rides along).
      Clusters are host-bin-packed into 8-per-core "windows" of <=16 clusters,
      each padded to exactly T_W tiles so PSUM accumulation windows are uniform
      across cores (one shared SPMD program).
  pass C (second launch): expand table rows back to per-node rows via a
      PE one-hot broadcast matmul, contiguous DMA out in sorted order.
Host does only O(C) table math, index bookkeeping, and the final unpermute.
"""

import sys

if "/opt/trn_rl_repo" not in sys.path:
    sys.path.insert(0, "/opt/trn_rl_repo")

import numpy as np
from contextlib import ExitStack

import concourse.bass as bass
import concourse.tile as tile
from concourse import bacc, mybir
from concourse.bass_utils import run_bass_kernel_spmd
from concourse.masks import make_identity

F32 = mybir.dt.float32

# Problem constants
N = 200000
HID = 256
HEADS = 8
HD = HID // HEADS
C = 1000
SCALE = HD ** -0.5

# Sharding constants
N_CORES = 8
TILE = 128
SLOTS = 16          # clusters per window
N_WIN = 8           # windows per core  (8*16*8 = 1024 cluster slots >= 1000)
T_W = 26            # tiles per window (padded)
NTILES = N_WIN * T_W           # 208 tiles per core
NL = NTILES * TILE             # 26624 nodes per core (padded)
PAD_SLOT = 255.0

EXPAND_ON_DEVICE = False
USE_F32R = True     # fp32r (fast fp32, ~1.5e-4 matmul rel err) for scores + U matmuls


# ----------------------------------------------------------------------------
# Device programs
# ----------------------------------------------------------------------------

def build_main_program(n_win=N_WIN, t_w=T_W, repeat=1, hw_loop=0):
    """Single-sweep program: per 128-node tile, transpose x on PE to get the
    feature-major view for the scores matmul, exp on ACT, build the weighted
    indicator on DVE, and accumulate U/z with one fp32 matmul. Only the
    node-major x is uploaded. `repeat` re-runs the whole sweep (for timing)."""
    ntiles = n_win * t_w
    nl = ntiles * TILE
    nc = bacc.Bacc("TRN2", target_bir_lowering=False, debug=False,
                   enable_asserts=False, num_devices=N_CORES)

    x_d = nc.dram_tensor("x", [nl, HID], F32, kind="ExternalInput").ap()
    segw_d = nc.dram_tensor("segw", [TILE, ntiles], F32, kind="ExternalInput").ap()
    win16_d = nc.dram_tensor("win16", [TILE, TILE], F32, kind="ExternalInput").ap()
    As_d = nc.dram_tensor("As", [HID, HEADS], F32, kind="ExternalInput").ap()
    c0_d = nc.dram_tensor("c0", [HEADS, 1], F32, kind="ExternalInput").ap()
    U_d = nc.dram_tensor("U", [TILE, n_win * HID], F32, kind="ExternalOutput").ap()
    z_d = nc.dram_tensor("z", [TILE, n_win], F32, kind="ExternalOutput").ap()

    with tile.TileContext(nc) as tc, ExitStack() as ctx:
        consts = ctx.enter_context(tc.tile_pool(name="consts", bufs=1))
        accs = ctx.enter_context(tc.tile_pool(name="accs", bufs=1))
        x_pool = ctx.enter_context(tc.tile_pool(name="xc", bufs=3))
        xt_pool = ctx.enter_context(tc.tile_pool(name="xt", bufs=3))
        xr_pool = ctx.enter_context(tc.tile_pool(name="xr", bufs=4))
        e_pool = ctx.enter_context(tc.tile_pool(name="et", bufs=3))
        mask_pool = ctx.enter_context(tc.tile_pool(name="mask", bufs=4))
        wind_pool = ctx.enter_context(tc.tile_pool(name="wind", bufs=4))
        xtp_psum = ctx.enter_context(tc.tile_pool(name="xtps", bufs=2, space="PSUM"))
        sc_psum = ctx.enter_context(tc.tile_pool(name="scps", bufs=2, space="PSUM"))
        et_psum = ctx.enter_context(tc.tile_pool(name="etps", bufs=2, space="PSUM"))
        uz_psum = ctx.enter_context(tc.tile_pool(name="uzps", bufs=2, space="PSUM"))

        F32X_c = mybir.dt.float32r if USE_F32R else F32
        win16_sb = consts.tile([TILE, TILE], F32)
        nc.sync.dma_start(win16_sb[:], win16_d[:])
        As_sb = consts.tile([TILE, 2 * HEADS], F32X_c)
        As_src = As_d[:].bitcast(F32X_c) if USE_F32R else As_d[:]
        nc.sync.dma_start(As_sb[:, 0:HEADS], As_src[0:TILE, :])
        nc.sync.dma_start(As_sb[:, HEADS:2 * HEADS], As_src[TILE:HID, :])
        c0_sb = consts.tile([HEADS, 1], F32)
        nc.sync.dma_start(c0_sb[:], c0_d[:])
        segw_sb = consts.tile([TILE, ntiles], F32)
        nc.sync.dma_start(segw_sb[:], segw_d[:])
        ident_sb = consts.tile([TILE, TILE], F32)
        make_identity(nc, ident_sb[:])

        U_sb = accs.tile([TILE, n_win * HID], F32)
        z_sb = accs.tile([TILE, n_win], F32)

        F32X = mybir.dt.float32r if USE_F32R else F32
        HA = HID + 2   # per-tile stride in the chunk: 256 x + ones col + pad col

        def sweep():
            uz_cur = None
            for chb in range(ntiles // 4):
                xc = x_pool.tile([TILE, 4 * HA], F32, tag="xc")
                xcv = xc[:].rearrange("p (a j) -> p a j", a=4)
                src = x_d[chb * 4 * TILE:(chb + 1) * 4 * TILE, :] \
                    .rearrange("(a p) j -> p a j", p=TILE)
                nc.sync.dma_start(xcv[:, :, 0:HID], src)
                nc.vector.memset(xcv[:, :, HID:HID + 2], 1.0)
                for pp in range(2):
                    # --- paired-tile scores: feature-major via PE transpose ---
                    xtp = xtp_psum.tile([TILE, 2 * HID], F32, tag="xtp")
                    for i in range(2):
                        xa = xc[:, (2 * pp + i) * HA:(2 * pp + i) * HA + HID]
                        nc.tensor.transpose(xtp[:, i * TILE:(i + 1) * TILE],
                                            xa[:, 0:TILE], ident_sb[:])
                        nc.tensor.transpose(
                            xtp[:, 2 * TILE + i * TILE:2 * TILE + (i + 1) * TILE],
                            xa[:, TILE:HID], ident_sb[:])
                    xt = xt_pool.tile([TILE, 2 * HID], F32X, tag="xt")
                    nc.scalar.copy(xt[:], xtp[:])
                    scp = sc_psum.tile([HEADS, 2 * TILE], F32, tag="scp")
                    nc.tensor.matmul(scp[:], lhsT=As_sb[:, 0:HEADS],
                                     rhs=xt[:, 0:2 * TILE],
                                     start=True, stop=False)
                    nc.tensor.matmul(scp[:], lhsT=As_sb[:, HEADS:2 * HEADS],
                                     rhs=xt[:, 2 * TILE:4 * TILE],
                                     start=False, stop=True)
                    e8 = e_pool.tile([HEADS, 2 * TILE], F32, tag="e8")
                    nc.scalar.activation(e8[:], scp[:],
                                         mybir.ActivationFunctionType.Exp,
                                         bias=c0_sb[:, 0:1], scale=1.0)
                    for i in range(2):
                        t = chb * 4 + 2 * pp + i
                        w, ti = divmod(t, t_w)
                        xa2 = xc[:, (2 * pp + i) * HA:(2 * pp + i + 1) * HA]
                        xr = xr_pool.tile([TILE, HA], F32X, tag="xr")
                        nc.vector.tensor_copy(xr[:], xa2)
                        etp = et_psum.tile([TILE, HEADS], F32, tag="etp")
                        nc.tensor.transpose(etp[:], e8[:, i * TILE:(i + 1) * TILE],
                                            ident_sb[0:HEADS, 0:HEADS])
                        et = e_pool.tile([TILE, HEADS], F32, tag="et")
                        nc.vector.tensor_copy(et[:], etp[:])
                        mask = mask_pool.tile([TILE, TILE], F32, tag="mask")
                        nc.gpsimd.tensor_scalar(mask[:], win16_sb[:],
                                                segw_sb[:, t:t + 1], None,
                                                op0=mybir.AluOpType.is_equal)
                        wind = wind_pool.tile([TILE, TILE], F32X, tag="wind")
                        e_rep = et[:].unsqueeze(1).to_broadcast(
                            [TILE, SLOTS, HEADS])
                        nc.vector.tensor_tensor(wind[:], mask[:], e_rep,
                                                op=mybir.AluOpType.mult)
                        if ti == 0:
                            uz_cur = uz_psum.tile([TILE, HA], F32, tag="uz")
                        nc.tensor.matmul(uz_cur[:], lhsT=wind[:], rhs=xr[:],
                                         start=(ti == 0), stop=(ti == t_w - 1))
                        if ti == t_w - 1:
                            nc.vector.tensor_copy(U_sb[:, w * HID:(w + 1) * HID],
                                                  uz_cur[:, 0:HID])
                            nc.vector.tensor_copy(z_sb[:, w:w + 1],
                                                  uz_cur[:, HID:HID + 1])

        if hw_loop > 1:
            with tc.For_i(0, hw_loop, 1):
                sweep()
        else:
            for _rep in range(repeat):
                sweep()

        nc.sync.dma_start(U_d[:], U_sb[:])
        nc.sync.dma_start(z_d[:], z_sb[:])

    nc.compile()
    return nc


def build_expand_program(n_win=N_WIN, t_w=T_W):
    """Pass C: out_sorted[n, :] = table[16*(t//T_W) + segw[n], :] via PE broadcast."""
    ntiles = n_win * t_w
    nl = ntiles * TILE
    nc = bacc.Bacc("TRN2", target_bir_lowering=False, debug=False,
                   enable_asserts=False, num_devices=N_CORES)

    table_d = nc.dram_tensor("table", [SLOTS, n_win * HID], F32,
                             kind="ExternalInput").ap()
    segw_d = nc.dram_tensor("segw", [TILE, ntiles], F32, kind="ExternalInput").ap()
    win16p_d = nc.dram_tensor("win16p", [TILE, SLOTS], F32,
                              kind="ExternalInput").ap()
    out_d = nc.dram_tensor("outs", [nl, HID], F32, kind="ExternalOutput").ap()

    with tile.TileContext(nc) as tc, ExitStack() as ctx:
        consts = ctx.enter_context(tc.tile_pool(name="consts", bufs=1))
        ind_pool = ctx.enter_context(tc.tile_pool(name="ind", bufs=4))
        indt_pool = ctx.enter_context(tc.tile_pool(name="indt", bufs=4))
        out_pool = ctx.enter_context(tc.tile_pool(name="outt", bufs=4))
        tp_psum = ctx.enter_context(tc.tile_pool(name="tpps", bufs=4, space="PSUM"))
        o_psum = ctx.enter_context(tc.tile_pool(name="ops", bufs=4, space="PSUM"))

        table_sb = consts.tile([SLOTS, n_win * HID], F32)
        nc.sync.dma_start(table_sb[:], table_d[:])
        segw_sb = consts.tile([TILE, ntiles], F32)
        nc.sync.dma_start(segw_sb[:], segw_d[:])
        win16p_sb = consts.tile([TILE, SLOTS], F32)
        nc.sync.dma_start(win16p_sb[:], win16p_d[:])
        ident_sb = consts.tile([TILE, TILE], F32)
        make_identity(nc, ident_sb[:])

        for t in range(ntiles):
            w = t // t_w
            ind = ind_pool.tile([TILE, SLOTS], F32, tag="ind")
            nc.vector.tensor_scalar(ind[:], win16p_sb[:], segw_sb[:, t:t + 1],
                                    None, op0=mybir.AluOpType.is_equal)
            tp = tp_psum.tile([SLOTS, TILE], F32, tag="tp")
            nc.tensor.transpose(tp[:], ind[:], ident_sb[:])
            indt = indt_pool.tile([SLOTS, TILE], F32, tag="indt")
            nc.vector.tensor_copy(indt[:], tp[:])
            ops = o_psum.tile([TILE, HID], F32, tag="ops")
            nc.tensor.matmul(ops[:], lhsT=indt[:],
                             rhs=table_sb[:, w * HID:(w + 1) * HID],
                             start=True, stop=True)
            outt = out_pool.tile([TILE, HID], F32, tag="outt")
            nc.vector.tensor_copy(outt[:], ops[:])
            nc.sync.dma_start(out_d[t * TILE:(t + 1) * TILE, :], outt[:])

    nc.compile()
    return nc


# ----------------------------------------------------------------------------
# Host-side planning
# ----------------------------------------------------------------------------

def plan_sharding(ca):
    """Bin-pack 1000 clusters into 64 (core, window) bins, <=16 clusters and
    <= T_W*128 nodes per bin. Returns per-core node index arrays + slot maps."""
    counts = np.bincount(ca, minlength=C)
    order = np.argsort(counts, kind="stable")[::-1]
    nbins = N_CORES * N_WIN
    cap = T_W * TILE
    loads = np.zeros(nbins, dtype=np.int64)
    nslots = np.zeros(nbins, dtype=np.int64)
    bin_clusters = [[] for _ in range(nbins)]
    for c in order:
        # least-loaded bin with a free slot
        cand = np.where(nslots < SLOTS)[0]
        b = cand[np.argmin(loads[cand])]
        bin_clusters[b].append(int(c))
        loads[b] += counts[c]
        nslots[b] += 1
    assert loads.max() <= cap, f"bin overflow: {loads.max()} > {cap}"

    # node lists per cluster (sorted order)
    idx_sorted = np.argsort(ca, kind="stable")
    starts = np.zeros(C + 1, dtype=np.int64)
    np.cumsum(counts, out=starts[1:])

    node_idx = np.full((N_CORES, NL), -1, dtype=np.int64)
    segw = np.full((N_CORES, NL), PAD_SLOT, dtype=np.float32)
    slot_cluster = np.full((N_CORES, N_WIN, SLOTS), -1, dtype=np.int64)
    for b in range(nbins):
        core, w = divmod(b, N_WIN)
        pos = w * cap
        for s, c in enumerate(bin_clusters[b]):
            slot_cluster[core, w, s] = c
            m = counts[c]
            node_idx[core, pos:pos + m] = idx_sorted[starts[c]:starts[c] + m]
            segw[core, pos:pos + m] = s
            pos += m
    return node_idx, segw, slot_cluster, counts


def host_table_math(U_all, z_all, slot_cluster, counts, Wv, bv, Wo, bo):
    """[8,128,2048] U + [8,128,8] z -> projected per-cluster table [C, HID]."""
    # U[core][row=s*8+h, col=w*256+j] ; z[core][row, w]
    U5 = U_all.reshape(N_CORES, SLOTS, HEADS, N_WIN, HID)
    z4 = z_all.reshape(N_CORES, SLOTS, HEADS, N_WIN)
    Uc = np.zeros((C, HEADS, HID), dtype=np.float64)
    zc = np.zeros((C, HEADS), dtype=np.float64)
    sc = slot_cluster  # [core, w, s]
    valid = sc >= 0
    cores, ws, ss = np.nonzero(valid)
    cl = sc[cores, ws, ss]
    Uc[cl] = U5[cores, ss, :, ws, :]
    zc[cl] = z4[cores, ss, :, ws]
    zc_safe = np.where(zc > 0, zc, 1.0)
    T = Uc / zc_safe[:, :, None]                      # [C, H, HID]
    Wv_r = np.asarray(Wv, np.float64).reshape(HEADS, HD, HID)
    ssum = np.einsum("chj,hdj->chd", T, Wv_r)         # [C, H, HD]
    ssum += np.asarray(bv, np.float64).reshape(HEADS, HD)[None]
    ssum = ssum.reshape(C, HID)
    ssum[counts == 0] = 0.0
    pooled = ssum / np.maximum(counts, 1)[:, None]
    table = pooled @ np.asarray(Wo, np.float64).T + np.asarray(bo, np.float64)
    return table.astype(np.float32)


_CACHE = {}


def make_runner(nc, n_cores=N_CORES):
    """Persistent jitted runner for a compiled Bacc program (axon/PJRT path).

    Same mechanism as run_bass_kernel_spmd's axon redirect (bass2jax), but the
    jitted executable is built once and reused, so steady-state calls skip
    retracing/lowering."""
    import jax
    from jax.sharding import Mesh, PartitionSpec, NamedSharding
    from jax.experimental.shard_map import shard_map
    from concourse.bass2jax import (_bass_exec_p, install_neuronx_cc_hook,
                                    partition_id_tensor)

    install_neuronx_cc_hook()
    in_names, out_names, out_avals = [], [], []
    partition_name = nc.partition_id_tensor.name if nc.partition_id_tensor else None
    for alloc in nc.m.functions[0].allocations:
        if not isinstance(alloc, mybir.MemoryLocationSet):
            continue
        name = alloc.memorylocations[0].name
        if alloc.kind == "ExternalInput":
            if name != partition_name:
                in_names.append(name)
        elif alloc.kind == "ExternalOutput":
            out_names.append(name)
            shape = tuple(alloc.tensor_shape)
            dtype = mybir.dt.np(alloc.dtype)
            out_avals.append(jax.core.ShapedArray(shape, dtype))
    n_params = len(in_names)
    n_outs = len(out_avals)
    all_in_names = list(in_names) + list(out_names)
    if partition_name:
        all_in_names.append(partition_name)

    def _body(*args):
        operands = list(args)
        if partition_name:
            operands.append(partition_id_tensor())
        return tuple(_bass_exec_p.bind(
            *operands, out_avals=tuple(out_avals), in_names=tuple(all_in_names),
            out_names=tuple(out_names), lowering_input_output_aliases=(),
            sim_require_finite=True, sim_require_nnan=True, nc=nc))

    devices = jax.devices()[:n_cores]
    mesh = Mesh(np.asarray(devices), ("core",))
    donate = tuple(range(n_params, n_params + n_outs))
    sharded = jax.jit(
        shard_map(_body, mesh=mesh,
                  in_specs=(PartitionSpec("core"),) * (n_params + n_outs),
                  out_specs=(PartitionSpec("core"),) * n_outs, check_rep=False),
        donate_argnums=donate, keep_unused=True)
    sharding = NamedSharding(mesh, PartitionSpec("core"))
    zero_shapes = [(n_cores * a.shape[0], *a.shape[1:]) for a in out_avals]
    zero_dtypes = [a.dtype for a in out_avals]

    def run(in_maps):
        import jax as _jax
        concat_in = [np.concatenate([np.asarray(m[name]) for m in in_maps],
                                    axis=0) for name in in_names]
        zs = [_jax.device_put(np.zeros(s, d), sharding)
              for s, d in zip(zero_shapes, zero_dtypes)]
        outs = _jax.block_until_ready(sharded(*concat_in, *zs))
        return [{name: np.asarray(outs[i]).reshape(n_cores, *out_avals[i].shape)[c]
                 for i, name in enumerate(out_names)}
                for c in range(n_cores)]

    return run


def _get_programs():
    if "main" not in _CACHE:
        _CACHE["main"] = build_main_program()
        _CACHE["main_run"] = make_runner(_CACHE["main"])
    if EXPAND_ON_DEVICE and "expand" not in _CACHE:
        _CACHE["expand"] = build_expand_program()
        _CACHE["expand_run"] = make_runner(_CACHE["expand"])
    return _CACHE


# ----------------------------------------------------------------------------
# Entry point
# ----------------------------------------------------------------------------

def kernel(x, cluster_assignments, batch, Wk, bk, Wv, bv, Wo, bo, pool_query):
    x = np.ascontiguousarray(np.asarray(x, dtype=np.float32))
    ca = np.asarray(cluster_assignments).astype(np.int64)
    Wk = np.asarray(Wk, np.float32)
    bk = np.asarray(bk, np.float32)
    pq = np.asarray(pool_query, np.float32)[0]  # [H, HD]

    # folded score projection
    As = (np.asarray(Wk, np.float64).reshape(HEADS, HD, HID)
          * np.asarray(pq, np.float64)[:, :, None]).sum(1)     # [H, HID]
    As = (As.T * SCALE).astype(np.float32)                     # [HID, H]
    c0 = ((np.asarray(bk, np.float64).reshape(HEADS, HD)
           * np.asarray(pq, np.float64)).sum(1) * SCALE).astype(np.float32)

    node_idx, segw, slot_cluster, counts = plan_sharding(ca)

    xpad = np.vstack([x, np.zeros((1, HID), np.float32)])
    nip = np.where(node_idx >= 0, node_idx, N)

    win16 = np.repeat(np.arange(SLOTS, dtype=np.float32), HEADS)[None, :] \
        .repeat(TILE, 0).copy()                                 # [128, 128]
    progs = _get_programs()

    in_maps = []
    for core in range(N_CORES):
        x_core = xpad[nip[core]]                                # [NL, HID]
        segw_core = np.ascontiguousarray(
            segw[core].reshape(NTILES, TILE).T)                 # [128, NTILES]
        in_maps.append({
            "x": x_core,
            "segw": segw_core,
            "win16": win16,
            "As": As,
            "c0": c0[:, None].copy(),
        })

    results = progs["main_run"](in_maps)
    U_all = np.stack([results[i]["U"] for i in range(N_CORES)])
    z_all = np.stack([results[i]["z"] for i in range(N_CORES)])

    table = host_table_math(U_all, z_all, slot_cluster, counts, Wv, bv, Wo, bo)

    out = np.empty((N, HID), dtype=np.float32)
    if EXPAND_ON_DEVICE:
        win16p = np.arange(SLOTS, dtype=np.float32)[None, :].repeat(TILE, 0).copy()
        # per-core local table [n_win*SLOTS, HID] in (w, s) order
        in_maps2 = []
        for core in range(N_CORES):
            tloc = np.zeros((N_WIN * SLOTS, HID), np.float32)
            sc = slot_cluster[core].reshape(-1)                 # [n_win*SLOTS]
            ok = sc >= 0
            tloc[ok] = table[sc[ok]]
            # [w*SLOTS+s, j] -> [s, w*HID+j]
            tloc = np.ascontiguousarray(
                tloc.reshape(N_WIN, SLOTS, HID).transpose(1, 0, 2)
                .reshape(SLOTS, N_WIN * HID))
            in_maps2.append({
                "table": tloc,
                "segw": in_maps[core]["segw"],
                "win16p": win16p,
            })
        results2 = progs["expand_run"](in_maps2)
        for core in range(N_CORES):
            sel = node_idx[core] >= 0
            out[node_idx[core][sel]] = results2[core]["outs"][sel]
    else:
        np.take(table, ca, axis=0, out=out)
    return out
